# revision 5
# baseline (speedup 1.0000x reference)
"""Trainium2 Bass kernel for LocalRelationalLayer (sparse_attention).

Computation (per reference):
  xp = zero-pad(x, 3)                                   # [B,256,62,62]
  km = 1x1conv(xp, k_w)+k_b ; qm = 1x1conv(xp, q_w)+q_b # [B,32,·,·]
  E[b,cm,l,ky,kx] = exp(km[b,cm,r+ky,w+kx]*qm[b,cm,r+3,w+3] + gpk[cm,ky,kx])
  ck = E / sum_kx E                                     # softmax over kx only
  pre[b,m*32+cm,l] = sum_{ky,kx} ck * xp[b,m*32+cm,r+ky,w+kx]
  out = 1x1conv(pre, f_w)+f_b                           # [B,256,56,56]

Sharding: 8 cores = (b in 2) x (4 row-blocks of 14 output rows). Every step is
core-local (halo of 3 rows is included in the per-core input slice); the host
concatenates the per-core outputs. No collectives.

Per-core layout: channel chunks of 128 on SBUF partitions. The key/query maps
(32 channels) are computed 4x-replicated along partitions by replicating k_w/q_w
in the matmul's stationary (M) dimension, so every weight-side elementwise op
directly matches the 128-channel value tiles (cm = c % 32).
All elementwise hot-path work is bf16 with even-offset access patterns (shifted
copies serve the odd kx taps) to keep the DVE in its 2x perf mode.
"""

import numpy as np
import ml_dtypes

B, C, H, W = 2, 256, 56, 56
K, PAD, M, CM = 7, 3, 8, 32
MID = CM // 2
HP, WP = H + 2 * PAD, W + 2 * PAD      # 62, 62
RB = 4                                  # row blocks per batch
RH = H // RB                            # 14 output rows per core
RHP = RH + K - 1                        # 20 padded rows per core
NCORES = 8
L = RH * W                              # 784 output positions per core

_bf16 = ml_dtypes.bfloat16
_PROGRAM = None


def _build_program():
    """Builds the Bass/Tile program (identical on all 8 cores)."""
    import concourse.bass as bass
    import concourse.tile as tile
    from concourse import bacc, mybir

    f32 = mybir.dt.float32
    bf16 = mybir.dt.bfloat16
    Exp = mybir.ActivationFunctionType.Exp
    Ln = mybir.ActivationFunctionType.Ln
    Ident = mybir.ActivationFunctionType.Identity
    PS = bass.MemorySpace.PSUM

    nc = bacc.Bacc("TRN2", target_bir_lowering=False, debug=False,
                   num_devices=NCORES)

    xp_d = nc.dram_tensor("xp", [2, 128, RHP, WP], bf16, kind="ExternalInput")
    wk_d = nc.dram_tensor("wk", [2, 128, 128], bf16, kind="ExternalInput")
    wq_d = nc.dram_tensor("wq", [2, 128, 128], bf16, kind="ExternalInput")
    fw_d = nc.dram_tensor("fw", [2, 2, 128, 128], bf16, kind="ExternalInput")
    gpk_d = nc.dram_tensor("gpk", [128, K * K], f32, kind="ExternalInput")
    kb_d = nc.dram_tensor("kb", [128, 1], f32, kind="ExternalInput")
    qb_d = nc.dram_tensor("qb", [128, 1], f32, kind="ExternalInput")
    fb_d = nc.dram_tensor("fb", [2, 128, 1], f32, kind="ExternalInput")
    y_d = nc.dram_tensor("y", [2, 128, RH, W], f32, kind="ExternalOutput")

    with tile.TileContext(nc) as tc:
        with (
            tc.tile_pool(name="inp", bufs=1) as inp,
            tc.tile_pool(name="wpool", bufs=1) as wpool,
            tc.tile_pool(name="kq", bufs=1) as kq,
            tc.tile_pool(name="ew", bufs=3) as ew,
            tc.tile_pool(name="pv", bufs=3) as pvp,
            tc.tile_pool(name="sm", bufs=4) as sm,
            tc.tile_pool(name="outp", bufs=1) as outp,
            tc.tile_pool(name="psum", bufs=4, space=PS) as psp,
        ):
            # ---- load inputs ----
            xv = []
            for c2 in range(2):
                t = inp.tile([128, RHP, WP], bf16, tag=f"xv{c2}", name=f"xv{c2}")
                nc.sync.dma_start(t[:], xp_d.ap()[c2])
                xv.append(t)
            wk = []
            wq = []
            for c2 in range(2):
                t = wpool.tile([128, 128], bf16, tag=f"wk{c2}", name=f"wk{c2}")
                nc.sync.dma_start(t[:], wk_d.ap()[c2])
                wk.append(t)
                t = wpool.tile([128, 128], bf16, tag=f"wq{c2}", name=f"wq{c2}")
                nc.sync.dma_start(t[:], wq_d.ap()[c2])
                wq.append(t)
            fw = [[None, None], [None, None]]
            for ci in range(2):
                for o in range(2):
                    t = wpool.tile([128, 128], bf16, tag=f"fw{ci}{o}", name=f"fw{ci}{o}")
                    nc.sync.dma_start(t[:], fw_d.ap()[ci][o])
                    fw[ci][o] = t
            gpk = wpool.tile([128, K * K], f32, tag="gpk", name="gpk")
            nc.sync.dma_start(gpk[:], gpk_d.ap())
            kb = wpool.tile([128, 1], f32, tag="kb", name="kb")
            nc.sync.dma_start(kb[:], kb_d.ap())
            qb = wpool.tile([128, 1], f32, tag="qb", name="qb")
            nc.sync.dma_start(qb[:], qb_d.ap())
            fb = []
            for o in range(2):
                t = wpool.tile([128, 1], f32, tag=f"fb{o}", name=f"fb{o}")
                nc.sync.dma_start(t[:], fb_d.ap()[o])
                fb.append(t)

            # ---- km (padded grid) and qm (center rows) via matmul ----
            # km[p, r', w'] for r' in [0,20), w' in [0,62): 4x-replicated rows.
            km = kq.tile([128, RHP, WP], bf16, tag="km", name="km")
            km_f = km[:].rearrange("p r w -> p (r w)")
            NKM = RHP * WP  # 1240
            for off in range(0, NKM, 512):
                n = min(512, NKM - off)
                ps = psp.tile([128, 512], f32, tag="ps", name="ps")
                for c2 in range(2):
                    rhs = xv[c2][:].rearrange("p r w -> p (r w)")[:, off:off + n]
                    nc.tensor.matmul(ps[:, :n], wk[c2][:], rhs,
                                     start=(c2 == 0), stop=(c2 == 1))
                nc.scalar.activation(km_f[:, off:off + n], ps[:, :n], Ident,
                                     bias=kb[:], scale=1.0)
            # qm on center rows 3..17, all 62 cols: [128, 14, 62]
            qm = kq.tile([128, RH, WP], bf16, tag="qm", name="qm")
            qm_f = qm[:].rearrange("p r w -> p (r w)")
            NQM = RH * WP  # 868
            for off in range(0, NQM, 512):
                n = min(512, NQM - off)
                ps = psp.tile([128, 512], f32, tag="ps", name="ps")
                for c2 in range(2):
                    rhs = xv[c2][:].rearrange("p r w -> p (r w)")[:, PAD * WP + off:
                                                                 PAD * WP + off + n]
                    nc.tensor.matmul(ps[:, :n], wq[c2][:], rhs,
                                     start=(c2 == 0), stop=(c2 == 1))
                nc.scalar.activation(qm_f[:, off:off + n], ps[:, :n], Ident,
                                     bias=qb[:], scale=1.0)

            # ---- odd-shifted copies (keep DVE 2x alignment for odd kx) ----
            km_o = kq.tile([128, RHP, WP], bf16, tag="km_o", name="km_o")
            nc.vector.tensor_copy(km_o[:].rearrange("p r w -> p (r w)")[:, 0:NKM - 1],
                                  km_f[:, 1:NKM])
            xv_o = []
            for c2 in range(2):
                t = kq.tile([128, RHP, WP], bf16, tag=f"xv_o{c2}", name=f"xv_o{c2}")
                nc.vector.tensor_copy(
                    t[:].rearrange("p r w -> p (r w)")[:, 0:NKM - 1],
                    xv[c2][:].rearrange("p r w -> p (r w)")[:, 1:NKM])
                xv_o.append(t)

            # center view of qm: [128, 14, 56] starting at col 3
            qmc = qm[:, :, PAD:PAD + W]

            pre = [outp.tile([128, RH, W], bf16, tag=f"pre{c2}", name=f"pre{c2}") for c2 in range(2)]

            # ---- main loop over ky ----
            for ky in range(K):
                # weight products P = km_shift * qm_center  -> [128, 7, 14, 56]
                P = ew.tile([128, K, RH, W], bf16, tag="P", name="P")
                for kx in range(K):
                    if kx % 2 == 0:
                        src = km[:, ky:ky + RH, kx:kx + W]
                    else:
                        src = km_o[:, ky:ky + RH, kx - 1:kx - 1 + W]
                    nc.vector.tensor_mul(P[:, kx], src, qmc)
                # E = exp(P + gpk)
                E = ew.tile([128, K, RH, W], bf16, tag="E", name="E")
                for kx in range(K):
                    j = ky * K + kx
                    nc.scalar.activation(E[:, kx], P[:, kx], Exp,
                                         bias=gpk[:, j:j + 1], scale=1.0)
                # d = sum_kx E ; keep all sum-chains off the DVE (it is the
                # bottleneck with the products); GPSIMD is otherwise idle
                eng_d = nc.gpsimd
                d = sm.tile([128, RH, W], bf16, tag="d", name="d")
                eng_d.tensor_add(d[:], E[:, 0], E[:, 1])
                for kx in range(2, K):
                    eng_d.tensor_add(d[:], d[:], E[:, kx])
                # r = 1/d via exp(-ln(d)) on ACT (vector.reciprocal is slow)
                lnd = sm.tile([128, RH, W], f32, tag="lnd", name="lnd")
                nc.scalar.activation(lnd[:], d[:], Ln, bias=0.0, scale=1.0)
                r = sm.tile([128, RH, W], bf16, tag="r", name="r")
                nc.scalar.activation(r[:], lnd[:], Exp, bias=0.0, scale=-1.0)

                # value aggregation per channel chunk
                for c2 in range(2):
                    PV = pvp.tile([128, K, RH, W], bf16, tag="PV", name="PV")
                    for kx in range(K):
                        if kx % 2 == 0:
                            src = xv[c2][:, ky:ky + RH, kx:kx + W]
                        else:
                            src = xv_o[c2][:, ky:ky + RH, kx - 1:kx - 1 + W]
                        nc.vector.tensor_mul(PV[:, kx], E[:, kx], src)
                    eng_s = nc.gpsimd if (2 * ky + c2) % 3 != 0 else nc.vector
                    acc = sm.tile([128, RH, W], bf16, tag="acc", name="acc")
                    eng_s.tensor_add(acc[:], PV[:, 0], PV[:, 1])
                    for kx in range(2, K):
                        eng_s.tensor_add(acc[:], acc[:], PV[:, kx])
                    # pre += acc * r
                    t = sm.tile([128, RH, W], bf16, tag="t", name="t")
                    if ky == 0:
                        nc.vector.tensor_mul(pre[c2][:], acc[:], r[:])
                    else:
                        nc.vector.tensor_mul(t[:], acc[:], r[:])
                        nc.vector.tensor_add(pre[c2][:], pre[c2][:], t[:])

            # ---- final 1x1 conv: y[o] = sum_ci fw[ci][o].T @ pre[ci] + fb[o] ----
            for o in range(2):
                y_sb = outp.tile([128, RH, W], f32, tag=f"y{o}", name=f"y{o}")
                y_f = y_sb[:].rearrange("p r w -> p (r w)")
                for off in range(0, L, 512):
                    n = min(512, L - off)
                    ps = psp.tile([128, 512], f32, tag="psf", name="psf")
                    for ci in range(2):
                        rhs = pre[ci][:].rearrange("p r w -> p (r w)")[:, off:off + n]
                        nc.tensor.matmul(ps[:, :n], fw[ci][o][:], rhs,
                                         start=(ci == 0), stop=(ci == 1))
                    nc.scalar.activation(y_f[:, off:off + n], ps[:, :n], Ident,
                                         bias=fb[o][:], scale=1.0)
                nc.sync.dma_start(y_d.ap()[o], y_sb[:])

    nc.compile()
    return nc


def _get_program():
    global _PROGRAM
    if _PROGRAM is None:
        _PROGRAM = _build_program()
    return _PROGRAM


def _gpk_host(gp_w1, gp_b1, gp_w2, gp_b2):
    """GeometryPrior on host (tiny: 49 positions through a 2->16->32 MLP)."""
    a = np.arange(-(K // 2), K // 2 + 1, dtype=np.float32)
    x_pos = np.broadcast_to(a[None, :], (K, K))
    y_pos = np.broadcast_to(a[::-1][:, None], (K, K))
    pos = np.stack([x_pos, y_pos], 0).astype(np.float32)          # [2,7,7]
    h1 = np.einsum('pij,mp->mij', pos, np.asarray(gp_w1, np.float32))
    h1 = np.maximum(h1 + np.asarray(gp_b1, np.float32)[:, None, None], 0.0)
    gpk = np.einsum('mij,cm->cij', h1, np.asarray(gp_w2, np.float32))
    gpk = gpk + np.asarray(gp_b2, np.float32)[:, None, None]      # [32,7,7]
    return gpk


def make_inputs(x, k_w, k_b, q_w, q_b, gp_w1, gp_b1, gp_w2, gp_b2, f_w, f_b):
    """Returns per-core input maps (list of 8 dicts)."""
    x = np.asarray(x, np.float32)
    xp = np.zeros((B, C, HP, WP), np.float32)
    xp[:, :, PAD:PAD + H, PAD:PAD + W] = x
    xp16 = xp.astype(_bf16)

    def rep128(w32):  # [32, C] -> lhsT chunks [2, 128(k), 128(m)]
        w = np.tile(np.asarray(w32, np.float32), (4, 1))   # [128, 256]
        return np.ascontiguousarray(w.T.reshape(2, 128, 128)).astype(_bf16)

    wk = rep128(k_w)
    wq = rep128(q_w)
    fw = np.ascontiguousarray(
        np.asarray(f_w, np.float32).T.reshape(2, 128, 2, 128).transpose(0, 2, 1, 3)
    ).astype(_bf16)                                        # [ci, o, k, m]
    gpk = _gpk_host(gp_w1, gp_b1, gp_w2, gp_b2).reshape(32, K * K)
    gpk128 = np.ascontiguousarray(np.tile(gpk, (4, 1))).astype(np.float32)
    kb = np.tile(np.asarray(k_b, np.float32), 4).reshape(128, 1)
    qb = np.tile(np.asarray(q_b, np.float32), 4).reshape(128, 1)
    fb = np.asarray(f_b, np.float32).reshape(2, 128, 1)

    in_maps = []
    for core in range(NCORES):
        b, rb = divmod(core, RB)
        sl = np.ascontiguousarray(
            xp16[b, :, rb * RH:rb * RH + RHP, :].reshape(2, 128, RHP, WP))
        in_maps.append({
            "xp": sl, "wk": wk, "wq": wq, "fw": fw, "gpk": gpk128,
            "kb": kb, "qb": qb, "fb": fb,
        })
    return in_maps


def assemble(results):
    out = np.empty((B, C, H, W), np.float32)
    for core in range(NCORES):
        b, rb = divmod(core, RB)
        y = np.asarray(results[core]["y"], np.float32)     # [2,128,14,56]
        out[b, :, rb * RH:(rb + 1) * RH, :] = y.reshape(C, RH, W)
    return out


def kernel(**inputs):
    from concourse import bass_utils
    nc = _get_program()
    in_maps = make_inputs(**inputs)
    res = bass_utils.run_bass_kernel_spmd(nc, in_maps, list(range(NCORES)))
    return assemble(res.results)


# revision 10
# speedup vs baseline: 695.8537x; 695.8537x over previous
"""Trainium2 Bass kernel for LocalRelationalLayer (sparse_attention).

Computation (per reference):
  xp = zero-pad(x, 3)                                   # [B,256,62,62]
  km = 1x1conv(xp, k_w)+k_b ; qm = 1x1conv(xp, q_w)+q_b # [B,32,·,·]
  E[b,cm,l,ky,kx] = exp(km[b,cm,r+ky,w+kx]*qm[b,cm,r+3,w+3] + gpk[cm,ky,kx])
  ck = E / sum_kx E                                     # softmax over kx only
  pre[b,m*32+cm,l] = sum_{ky,kx} ck * xp[b,m*32+cm,r+ky,w+kx]
  out = 1x1conv(pre, f_w)+f_b                           # [B,256,56,56]

Sharding: 8 cores = (b in 2) x (4 row-blocks of 14 output rows). Every step is
core-local (halo of 3 rows is included in the per-core input slice); the host
concatenates the per-core outputs. No collectives.

Per-core layout: channel chunks of 128 on SBUF partitions. The key/query maps
(32 channels) are computed 4x-replicated along partitions by replicating k_w/q_w
in the matmul's stationary (M) dimension, so every weight-side elementwise op
directly matches the 128-channel value tiles (cm = c % 32).
All elementwise hot-path work is bf16 with even-offset access patterns (shifted
copies serve the odd kx taps) to keep the DVE in its 2x perf mode.
"""

import numpy as np
import ml_dtypes

B, C, H, W = 2, 256, 56, 56
K, PAD, M, CM = 7, 3, 8, 32
MID = CM // 2
HP, WP = H + 2 * PAD, W + 2 * PAD      # 62, 62
RB = 4                                  # row blocks per batch
RH = H // RB                            # 14 output rows per core
RHP = RH + K - 1                        # 20 padded rows per core
NCORES = 8
L = RH * W                              # 784 output positions per core

_bf16 = ml_dtypes.bfloat16
_PROGRAM = None


def _build_program():
    """Builds the Bass/Tile program (identical on all 8 cores)."""
    import concourse.bass as bass
    import concourse.tile as tile
    from concourse import bacc, mybir

    f32 = mybir.dt.float32
    bf16 = mybir.dt.bfloat16
    Exp = mybir.ActivationFunctionType.Exp
    Ln = mybir.ActivationFunctionType.Ln
    Ident = mybir.ActivationFunctionType.Identity
    PS = bass.MemorySpace.PSUM

    nc = bacc.Bacc("TRN2", target_bir_lowering=False, debug=False,
                   num_devices=NCORES)

    xp_d = nc.dram_tensor("xp", [2, 128, RHP, WP], bf16, kind="ExternalInput")
    wk_d = nc.dram_tensor("wk", [2, 128, 128], bf16, kind="ExternalInput")
    wq_d = nc.dram_tensor("wq", [2, 128, 128], bf16, kind="ExternalInput")
    fw_d = nc.dram_tensor("fw", [2, 2, 128, 128], bf16, kind="ExternalInput")
    gpk_d = nc.dram_tensor("gpk", [128, K * K], f32, kind="ExternalInput")
    kb_d = nc.dram_tensor("kb", [128, 1], f32, kind="ExternalInput")
    qb_d = nc.dram_tensor("qb", [128, 1], f32, kind="ExternalInput")
    fb_d = nc.dram_tensor("fb", [2, 128, 1], f32, kind="ExternalInput")
    y_d = nc.dram_tensor("y", [2, 128, RH, W], f32, kind="ExternalOutput")

    with tile.TileContext(nc) as tc:
        with (
            tc.tile_pool(name="inp", bufs=1) as inp,
            tc.tile_pool(name="wpool", bufs=1) as wpool,
            tc.tile_pool(name="kq", bufs=1) as kq,
            tc.tile_pool(name="ew", bufs=3) as ew,
            tc.tile_pool(name="pv", bufs=3) as pvp,
            tc.tile_pool(name="sm", bufs=4) as sm,
            tc.tile_pool(name="outp", bufs=1) as outp,
            tc.tile_pool(name="psum", bufs=4, space=PS) as psp,
        ):
            # ---- load inputs ----
            xv = []
            for c2 in range(2):
                t = inp.tile([128, RHP, WP], bf16, tag=f"xv{c2}", name=f"xv{c2}")
                nc.sync.dma_start(t[:], xp_d.ap()[c2])
                xv.append(t)
            wk = []
            wq = []
            for c2 in range(2):
                t = wpool.tile([128, 128], bf16, tag=f"wk{c2}", name=f"wk{c2}")
                nc.sync.dma_start(t[:], wk_d.ap()[c2])
                wk.append(t)
                t = wpool.tile([128, 128], bf16, tag=f"wq{c2}", name=f"wq{c2}")
                nc.sync.dma_start(t[:], wq_d.ap()[c2])
                wq.append(t)
            fw = [[None, None], [None, None]]
            for ci in range(2):
                for o in range(2):
                    t = wpool.tile([128, 128], bf16, tag=f"fw{ci}{o}", name=f"fw{ci}{o}")
                    nc.sync.dma_start(t[:], fw_d.ap()[ci][o])
                    fw[ci][o] = t
            gpk = wpool.tile([128, K * K], f32, tag="gpk", name="gpk")
            nc.sync.dma_start(gpk[:], gpk_d.ap())
            kb = wpool.tile([128, 1], f32, tag="kb", name="kb")
            nc.sync.dma_start(kb[:], kb_d.ap())
            qb = wpool.tile([128, 1], f32, tag="qb", name="qb")
            nc.sync.dma_start(qb[:], qb_d.ap())
            fb = []
            for o in range(2):
                t = wpool.tile([128, 1], f32, tag=f"fb{o}", name=f"fb{o}")
                nc.sync.dma_start(t[:], fb_d.ap()[o])
                fb.append(t)

            # ---- km (padded grid) and qm (center rows) via matmul ----
            # km[p, r', w'] for r' in [0,20), w' in [0,62): 4x-replicated rows.
            km = kq.tile([128, RHP, WP], bf16, tag="km", name="km")
            km_f = km[:].rearrange("p r w -> p (r w)")
            NKM = RHP * WP  # 1240
            for off in range(0, NKM, 512):
                n = min(512, NKM - off)
                ps = psp.tile([128, 512], f32, tag="ps", name="ps")
                for c2 in range(2):
                    rhs = xv[c2][:].rearrange("p r w -> p (r w)")[:, off:off + n]
                    nc.tensor.matmul(ps[:, :n], wk[c2][:], rhs,
                                     start=(c2 == 0), stop=(c2 == 1))
                nc.scalar.activation(km_f[:, off:off + n], ps[:, :n], Ident,
                                     bias=kb[:], scale=1.0)
            # qm on center rows 3..17, all 62 cols: [128, 14, 62]
            qm = kq.tile([128, RH, WP], bf16, tag="qm", name="qm")
            qm_f = qm[:].rearrange("p r w -> p (r w)")
            NQM = RH * WP  # 868
            for off in range(0, NQM, 512):
                n = min(512, NQM - off)
                ps = psp.tile([128, 512], f32, tag="ps", name="ps")
                for c2 in range(2):
                    rhs = xv[c2][:].rearrange("p r w -> p (r w)")[:, PAD * WP + off:
                                                                 PAD * WP + off + n]
                    nc.tensor.matmul(ps[:, :n], wq[c2][:], rhs,
                                     start=(c2 == 0), stop=(c2 == 1))
                nc.scalar.activation(qm_f[:, off:off + n], ps[:, :n], Ident,
                                     bias=qb[:], scale=1.0)

            # ---- odd-shifted copies (keep DVE 2x alignment for odd kx) ----
            km_o = kq.tile([128, RHP, WP], bf16, tag="km_o", name="km_o")
            nc.scalar.copy(km_o[:].rearrange("p r w -> p (r w)")[:, 0:NKM - 1],
                           km_f[:, 1:NKM])
            xv_o = []
            for c2 in range(2):
                t = kq.tile([128, RHP, WP], bf16, tag=f"xv_o{c2}", name=f"xv_o{c2}")
                nc.scalar.copy(
                    t[:].rearrange("p r w -> p (r w)")[:, 0:NKM - 1],
                    xv[c2][:].rearrange("p r w -> p (r w)")[:, 1:NKM])
                xv_o.append(t)

            # center view of qm: [128, 14, 56] starting at col 3
            qmc = qm[:, :, PAD:PAD + W]

            pre = [outp.tile([128, RH, W], bf16, tag=f"pre{c2}", name=f"pre{c2}") for c2 in range(2)]

            # ---- main loop over ky ----
            for ky in range(K):
                # weight products P = km_shift * qm_center  -> [128, 7, 14, 56]
                P = ew.tile([128, K, RH, W], bf16, tag="P", name="P")
                for kx in range(K):
                    if kx % 2 == 0:
                        src = km[:, ky:ky + RH, kx:kx + W]
                    else:
                        src = km_o[:, ky:ky + RH, kx - 1:kx - 1 + W]
                    nc.vector.tensor_mul(P[:, kx], src, qmc)
                # E = exp(P + gpk)
                E = ew.tile([128, K, RH, W], bf16, tag="E", name="E")
                for kx in range(K):
                    j = ky * K + kx
                    nc.scalar.activation(E[:, kx], P[:, kx], Exp,
                                         bias=gpk[:, j:j + 1], scale=1.0)
                # d = sum_kx E ; keep all sum-chains off the DVE (it is the
                # bottleneck with the products); GPSIMD is otherwise idle
                eng_d = nc.gpsimd
                d = sm.tile([128, RH, W], bf16, tag="d", name="d")
                eng_d.tensor_add(d[:], E[:, 0], E[:, 1])
                for kx in range(2, K):
                    eng_d.tensor_add(d[:], d[:], E[:, kx])
                # r = 1/d via exp(-ln(d)) on ACT (vector.reciprocal is slow)
                lnd = sm.tile([128, RH, W], f32, tag="lnd", name="lnd")
                nc.scalar.activation(lnd[:], d[:], Ln, bias=0.0, scale=1.0)
                r = sm.tile([128, RH, W], bf16, tag="r", name="r")
                nc.scalar.activation(r[:], lnd[:], Exp, bias=0.0, scale=-1.0)

                # value aggregation per channel chunk
                for c2 in range(2):
                    PV = pvp.tile([128, K, RH, W], bf16, tag="PV", name="PV")
                    eng_m = nc.vector
                    for kx in range(K):
                        if kx % 2 == 0:
                            src = xv[c2][:, ky:ky + RH, kx:kx + W]
                        else:
                            src = xv_o[c2][:, ky:ky + RH, kx - 1:kx - 1 + W]
                        eng_m.tensor_mul(PV[:, kx], E[:, kx], src)
                    eng_s = nc.gpsimd if (4 * ky + c2) % 5 != 0 else nc.vector
                    acc = sm.tile([128, RH, W], bf16, tag="acc", name="acc")
                    eng_s.tensor_add(acc[:], PV[:, 0], PV[:, 1])
                    for kx in range(2, K):
                        eng_s.tensor_add(acc[:], acc[:], PV[:, kx])
                    # pre += acc * r (mult on DVE; the serial accumulate-chain
                    # into pre alternates engines by chunk to avoid queueing)
                    t = sm.tile([128, RH, W], bf16, tag="t", name="t")
                    if ky == 0:
                        nc.vector.tensor_mul(pre[c2][:], acc[:], r[:])
                    else:
                        nc.vector.tensor_mul(t[:], acc[:], r[:])
                        eng_p = nc.gpsimd if c2 == 0 else nc.vector
                        eng_p.tensor_add(pre[c2][:], pre[c2][:], t[:])

            # ---- final 1x1 conv: y[o] = sum_ci fw[ci][o].T @ pre[ci] + fb[o] ----
            for o in range(2):
                y_sb = outp.tile([128, RH, W], f32, tag=f"y{o}", name=f"y{o}")
                y_f = y_sb[:].rearrange("p r w -> p (r w)")
                for off in range(0, L, 512):
                    n = min(512, L - off)
                    ps = psp.tile([128, 512], f32, tag="psf", name="psf")
                    for ci in range(2):
                        rhs = pre[ci][:].rearrange("p r w -> p (r w)")[:, off:off + n]
                        nc.tensor.matmul(ps[:, :n], fw[ci][o][:], rhs,
                                         start=(ci == 0), stop=(ci == 1))
                    nc.scalar.activation(y_f[:, off:off + n], ps[:, :n], Ident,
                                         bias=fb[o][:], scale=1.0)
                nc.sync.dma_start(y_d.ap()[o], y_sb[:])

    nc.compile()
    return nc


def _get_program():
    global _PROGRAM
    if _PROGRAM is None:
        _PROGRAM = _build_program()
    return _PROGRAM


def _gpk_host(gp_w1, gp_b1, gp_w2, gp_b2):
    """GeometryPrior on host (tiny: 49 positions through a 2->16->32 MLP)."""
    a = np.arange(-(K // 2), K // 2 + 1, dtype=np.float32)
    x_pos = np.broadcast_to(a[None, :], (K, K))
    y_pos = np.broadcast_to(a[::-1][:, None], (K, K))
    pos = np.stack([x_pos, y_pos], 0).astype(np.float32)          # [2,7,7]
    h1 = np.einsum('pij,mp->mij', pos, np.asarray(gp_w1, np.float32))
    h1 = np.maximum(h1 + np.asarray(gp_b1, np.float32)[:, None, None], 0.0)
    gpk = np.einsum('mij,cm->cij', h1, np.asarray(gp_w2, np.float32))
    gpk = gpk + np.asarray(gp_b2, np.float32)[:, None, None]      # [32,7,7]
    return gpk


def make_inputs(x, k_w, k_b, q_w, q_b, gp_w1, gp_b1, gp_w2, gp_b2, f_w, f_b):
    """Returns per-core input maps (list of 8 dicts)."""
    x = np.asarray(x, np.float32)
    xp = np.zeros((B, C, HP, WP), np.float32)
    xp[:, :, PAD:PAD + H, PAD:PAD + W] = x
    xp16 = xp.astype(_bf16)

    def rep128(w32):  # [32, C] -> lhsT chunks [2, 128(k), 128(m)]
        w = np.tile(np.asarray(w32, np.float32), (4, 1))   # [128, 256]
        return np.ascontiguousarray(w.T.reshape(2, 128, 128)).astype(_bf16)

    wk = rep128(k_w)
    wq = rep128(q_w)
    fw = np.ascontiguousarray(
        np.asarray(f_w, np.float32).T.reshape(2, 128, 2, 128).transpose(0, 2, 1, 3)
    ).astype(_bf16)                                        # [ci, o, k, m]
    gpk = _gpk_host(gp_w1, gp_b1, gp_w2, gp_b2).reshape(32, K * K)
    gpk128 = np.ascontiguousarray(np.tile(gpk, (4, 1))).astype(np.float32)
    kb = np.tile(np.asarray(k_b, np.float32), 4).reshape(128, 1)
    qb = np.tile(np.asarray(q_b, np.float32), 4).reshape(128, 1)
    fb = np.asarray(f_b, np.float32).reshape(2, 128, 1)

    in_maps = []
    for core in range(NCORES):
        b, rb = divmod(core, RB)
        sl = np.ascontiguousarray(
            xp16[b, :, rb * RH:rb * RH + RHP, :].reshape(2, 128, RHP, WP))
        in_maps.append({
            "xp": sl, "wk": wk, "wq": wq, "fw": fw, "gpk": gpk128,
            "kb": kb, "qb": qb, "fb": fb,
        })
    return in_maps


def assemble(results):
    out = np.empty((B, C, H, W), np.float32)
    for core in range(NCORES):
        b, rb = divmod(core, RB)
        y = np.asarray(results[core]["y"], np.float32)     # [2,128,14,56]
        out[b, :, rb * RH:(rb + 1) * RH, :] = y.reshape(C, RH, W)
    return out


def kernel(**inputs):
    from concourse import bass_utils
    nc = _get_program()
    in_maps = make_inputs(**inputs)
    res = bass_utils.run_bass_kernel_spmd(nc, in_maps, list(range(NCORES)))
    return assemble(res.results)
